# revision 1
# baseline (speedup 1.0000x reference)
"""EvolveGCNO distributed Bass kernel for TRN2 (8 cores).

Three-phase pipeline:
  A: per-core (natural node range) degree + dinv + z = dinv * x
  B: per-core (src-window x dst-half) edge gather/scale/accumulate partials
  C: per-core (natural node range) combine partials, dinv scale, GRU-evolved
     weight W, fused (W @ projW.T) matmul, relu, linear head.

Host does only integer index manipulation / permutation of arrays between
phases; all floating point math runs on device.
"""
import dataclasses
import numpy as np

import concourse.bass as bass
import concourse.bacc as bacc
import concourse.mybir as mybir
from concourse import library_config

F = 64
NC = 8
NWIN = 4
NHALF = 2
P = 128


def bcast_free(ap, k):
    """Append a stride-0 innermost free dim of size k to an AP (broadcast)."""
    return dataclasses.replace(ap, ap=list(ap.ap) + [[0, k]])


@dataclasses.dataclass
class Cfg:
    N: int = 100000
    CHUNK_SLOTS: int = 4096   # psum half: CHUNK_SLOTS*F*4 bytes = 1MB (4 banks)
    CALLTOK: int = 1024       # max tokens per dma_gather call (single_packet limit)
    NBUF: int = 3             # gather tile buffers

    @property
    def NPC(self):
        return self.N // NC

    @property
    def GP(self):
        return (self.NPC + P - 1) // P

    @property
    def NPCP(self):
        return self.GP * P

    @property
    def WINN(self):
        return self.N // NWIN

    @property
    def HALFN(self):
        return self.N // NHALF


# ---------------------------------------------------------------------------
# Host-side graph preprocessing (integer only)
# ---------------------------------------------------------------------------

def wrap_idx16(tokens):
    t = np.asarray(tokens, dtype=np.int16)
    assert t.size % 16 == 0
    arr = t.reshape(-1, 16).T
    return np.ascontiguousarray(np.tile(arr, (8, 1)))


def wrap_wv(wvals):
    w = np.asarray(wvals, dtype=np.float32)
    assert w.size % P == 0
    return np.ascontiguousarray(w.reshape(-1, P).T)


def grid_layout(vals_2d, cfg):
    """[NPCP, D] -> [128, GP*D]: node local g*128+p -> [p, g*D:(g+1)*D]."""
    D = vals_2d.shape[1]
    g = vals_2d.reshape(cfg.GP, P, D).transpose(1, 0, 2).reshape(P, cfg.GP * D)
    return np.ascontiguousarray(g)


def ungrid_layout(arr, cfg, D):
    return arr.reshape(P, cfg.GP, D).transpose(1, 0, 2).reshape(cfg.NPCP, D)


def prep_A(src, dst, w, cfg):
    N = cfg.N
    deg = np.bincount(dst, minlength=N)
    RA = max(int(deg.max()), 1)
    order = np.argsort(dst, kind="stable")
    dsts = dst[order]
    ws = w[order]
    starts = np.zeros(N + 1, dtype=np.int64)
    np.cumsum(np.bincount(dsts, minlength=N), out=starts[1:])
    rank = np.arange(len(dsts)) - starts[dsts]
    wfull = np.zeros((N, RA), dtype=np.float32)
    wfull[dsts, rank] = ws
    out = []
    for c in range(NC):
        blk = np.zeros((cfg.NPCP, RA), dtype=np.float32)
        blk[: cfg.NPC] = wfull[c * cfg.NPC : (c + 1) * cfg.NPC]
        out.append(grid_layout(blk, cfg))
    return RA, out


def prep_B(src, dst, w, cfg):
    N = cfg.N
    win = (src % NWIN).astype(np.int64)
    half = ((dst // NWIN) % NHALF).astype(np.int64)
    core_of_edge = half * NWIN + win
    tloc_src = (src // NWIN).astype(np.int64)

    vv = np.arange(N, dtype=np.int64)
    self_core = ((vv // NWIN) % NHALF) * NWIN + (vv % NWIN)

    SLOTS = ((cfg.HALFN + cfg.CHUNK_SLOTS - 1) // cfg.CHUNK_SLOTS) * cfg.CHUNK_SLOTS
    NCH = SLOTS // cfg.CHUNK_SLOTS

    per_core = []
    for k in range(NHALF):
        half_nodes = vv[(vv // NWIN) % NHALF == k]
        node_hidx = np.full(N, -1, dtype=np.int64)
        node_hidx[half_nodes] = np.arange(len(half_nodes))
        for wno in range(NWIN):
            cid = k * NWIN + wno
            em = core_of_edge == cid
            sm = self_core == cid
            s_v = vv[sm]
            tok_dst = np.concatenate([dst[em], s_v])
            tok_idx = np.concatenate([tloc_src[em], s_v // NWIN])
            tok_w = np.concatenate([w[em].astype(np.float32),
                                    np.ones(len(s_v), np.float32)])

            hidx = node_hidx[tok_dst]
            cnt = np.bincount(hidx, minlength=len(half_nodes))
            slot_order = np.argsort(-cnt, kind="stable")
            node_slot = np.empty(len(half_nodes), dtype=np.int64)
            node_slot[slot_order] = np.arange(len(half_nodes))
            cnt_sorted = cnt[slot_order]

            tok_slot = node_slot[hidx]
            order = np.argsort(tok_slot, kind="stable")
            ts = tok_slot[order]
            starts = np.zeros(len(half_nodes) + 1, dtype=np.int64)
            np.cumsum(np.bincount(ts, minlength=len(half_nodes)), out=starts[1:])
            rank = np.arange(len(ts)) - starts[ts]

            per_core.append(dict(
                cid=cid, half=k, win=wno,
                half_nodes=half_nodes, node_slot=node_slot, cnt_sorted=cnt_sorted,
                tok_slot=ts, tok_rank=rank,
                tok_idx=tok_idx[order], tok_w=tok_w[order],
            ))

    per_core.sort(key=lambda d: d["cid"])
    RMAX = max(int(pc["cnt_sorted"][0]) if len(pc["cnt_sorted"]) else 1
               for pc in per_core)

    njr = np.zeros((NCH, RMAX), dtype=np.int64)
    for pc in per_core:
        cs = np.zeros(SLOTS, dtype=np.int64)
        cs[: len(pc["cnt_sorted"])] = pc["cnt_sorted"]
        csg = cs.reshape(NCH, cfg.CHUNK_SLOTS)
        for r in range(RMAX):
            njr[:, r] = np.maximum(njr[:, r], (csg > r).sum(axis=1))
    njr[:, 0] = cfg.CHUNK_SLOTS
    njr = ((njr + P - 1) // P) * P

    segments = []
    for j in range(NCH):
        for r in range(RMAX):
            n = int(njr[j, r])
            if n > 0:
                segments.append((j, r, n))

    calls = []
    cur, cur_tok = [], 0
    for (j, r, n) in segments:
        if r == 0 and cur:
            calls.append(cur)          # round 0 must start a fresh call so its
            cur, cur_tok = [], 0       # psum start=True pieces are bank-aligned
        roff = 0
        while roff < n:
            take = min(n - roff, cfg.CALLTOK - cur_tok)
            cur.append((j, r, roff, take, cur_tok))
            cur_tok += take
            roff += take
            if cur_tok == cfg.CALLTOK:
                calls.append(cur)
                cur, cur_tok = [], 0
    if cur:
        calls.append(cur)
    TB = sum(n for (_, _, n) in segments)

    idx_arrs, wv_arrs = [], []
    for pc in per_core:
        gi = np.zeros((SLOTS, RMAX), dtype=np.int16)
        gw = np.zeros((SLOTS, RMAX, 2), dtype=np.float32)
        gi[pc["tok_slot"], pc["tok_rank"]] = (pc["tok_idx"] // 2).astype(np.int16)
        par = (pc["tok_idx"] % 2).astype(np.int64)
        gw[pc["tok_slot"], pc["tok_rank"], par] = pc["tok_w"]
        idx_stream = np.concatenate(
            [gi[j * cfg.CHUNK_SLOTS : j * cfg.CHUNK_SLOTS + n, r]
             for (j, r, n) in segments])
        wv_stream = np.concatenate(
            [gw[j * cfg.CHUNK_SLOTS : j * cfg.CHUNK_SLOTS + n, r]
             for (j, r, n) in segments])
        assert idx_stream.size == TB
        idx_arrs.append(wrap_idx16(idx_stream))
        # token t -> [t%128, t//128, 0:2]
        wv_arrs.append(np.ascontiguousarray(
            wv_stream.reshape(-1, P, 2).transpose(1, 0, 2).reshape(P, -1)))

    return dict(per_core=per_core, njr=njr, segments=segments, calls=calls,
                TB=TB, SLOTS=SLOTS, NCH=NCH, RMAX=RMAX,
                idx_arrs=idx_arrs, wv_arrs=wv_arrs)


# ---------------------------------------------------------------------------
# Bass programs
# ---------------------------------------------------------------------------

def build_A(cfg, RA, reps=1):
    nc = bacc.Bacc("TRN2")
    GP = cfg.GP
    xA = nc.dram_tensor("xA", [P, GP * F], mybir.dt.float32, kind="ExternalInput")
    wA = nc.dram_tensor("wA", [P, GP * RA], mybir.dt.float32, kind="ExternalInput")
    zA = nc.dram_tensor("zA", [P, GP * F], mybir.dt.float32, kind="ExternalOutput")
    dA = nc.dram_tensor("dA", [P, GP], mybir.dt.float32, kind="ExternalOutput")

    with (
        nc.sbuf_tensor("X", [P, GP, F], mybir.dt.float32) as X,
        nc.sbuf_tensor("WT", [P, GP, RA], mybir.dt.float32) as WT,
        nc.sbuf_tensor("DEG", [P, GP], mybir.dt.float32) as DEG,
        nc.sbuf_tensor("DINV", [P, GP], mybir.dt.float32) as DINV,
        nc.sbuf_tensor("Z", [P, GP, F], mybir.dt.float32) as Z,
        nc.semaphore("dma") as dma,
        nc.semaphore("zod") as zod,
        nc.semaphore("dod") as dod,
        nc.semaphore("ve") as ve,
        nc.semaphore("ac") as ac,
        nc.Block() as block,
    ):
        @block.sync
        def _(sync):
            for r in range(reps):
                if r > 0:
                    sync.wait_ge(ve, 2 * r)       # X/WT consumed by prev rep
                sync.dma_start(X[:, :, :], xA.rearrange("p (g f) -> p g f", f=F)).then_inc(dma, 16)
                sync.dma_start(WT[:, :, :], wA.rearrange("p (g r) -> p g r", r=RA)).then_inc(dma, 16)
                sync.wait_ge(ve, 2 * r + 2)
                sync.dma_start(zA[:, :], Z[:, :, :].rearrange("p g f -> p (g f)")).then_inc(zod, 16)
                sync.wait_ge(ac, r + 1)
                sync.dma_start(dA[:, :], DINV[:, :]).then_inc(dod, 16)
            sync.wait_ge(dma, 32 * reps)
            sync.wait_ge(zod, 16 * reps)
            sync.wait_ge(dod, 16 * reps)

        @block.vector
        def _(vector):
            for r in range(reps):
                vector.wait_ge(dma, 32 * r + 32)
                vector.tensor_reduce(DEG[:, :], WT[:, :, :], mybir.AxisListType.X,
                                     mybir.AluOpType.add)
                vector.drain()
                vector.tensor_scalar_add(DEG[:, :], DEG[:, :], 1.0)
                vector.drain()
                vector.reciprocal(DEG[:, :], DEG[:, :])
                vector.drain().then_inc(ve, 1)
                vector.wait_ge(ac, r + 1)
                if r > 0:
                    vector.wait_ge(zod, 16 * r)   # prev zA out done
                vector.tensor_tensor(Z[:, :, :], X[:, :, :], bcast_free(DINV[:, :], F),
                                     mybir.AluOpType.mult)
                vector.drain().then_inc(ve, 1)

        @block.scalar
        def _(scalar):
            for r in range(reps):
                scalar.wait_ge(ve, 2 * r + 1)
                if r > 0:
                    scalar.wait_ge(dod, 16 * r)   # prev dA out done
                scalar.sqrt(DINV[:, :], DEG[:, :])
                scalar.drain().then_inc(ac, 1)

    nc.compile()
    return nc


def build_B(cfg, sched, reps=1):
    nc = bacc.Bacc("TRN2")
    TB, SLOTS, NCH = sched["TB"], sched["SLOTS"], sched["NCH"]
    calls = sched["calls"]
    CHCOLS = cfg.CHUNK_SLOTS * F // P
    assert CHCOLS <= 2048
    SL_COLS = SLOTS * F // P
    njr = sched["njr"]
    RMAX = sched["RMAX"]
    NBANK = (CHCOLS + 511) // 512
    NBUF = cfg.NBUF
    CT = cfg.CALLTOK

    last_touch = {}
    for j in range(NCH):
        for b in range(NBANK):
            lt = 0
            for r in range(RMAX):
                if njr[j, r] * F // P > 512 * b:
                    lt = r
            last_touch[(j, b)] = lt
    chunk_last_r = {}
    for (j, r, n) in sched["segments"]:
        chunk_last_r[j] = r

    ztab = nc.dram_tensor("ztab", [cfg.WINN, F], mybir.dt.float32, kind="ExternalInput")
    idxB = nc.dram_tensor("idxB", [P, TB // 16], mybir.dt.int16, kind="ExternalInput")
    wvB = nc.dram_tensor("wvB", [P, TB // P * 2], mybir.dt.float32, kind="ExternalInput")
    idh = nc.dram_tensor("idh", [P, P], mybir.dt.float32, kind="ExternalInput")
    accB = nc.dram_tensor("accB", [P, SL_COLS], mybir.dt.float32, kind="ExternalOutput")

    call_off = []
    o = 0
    for call in calls:
        call_off.append(o)
        o += sum(n for (_, _, _, n, _) in call)
    assert o == TB

    with (
        nc.sbuf_tensor("G", [P, NBUF, CT // P, 2 * F], mybir.dt.float32) as G,
        nc.sbuf_tensor("IDX", [P, TB // 16], mybir.dt.int16) as IDX,
        nc.sbuf_tensor("WV", [P, TB // P, 2], mybir.dt.float32) as WV,
        nc.sbuf_tensor("ID", [P, P], mybir.dt.float32) as ID,
        nc.sbuf_tensor("OUT", [P, 2, CHCOLS], mybir.dt.float32) as OUT,
        nc.psum_tensor("PS", [P, 2, CHCOLS], mybir.dt.float32) as PS,
        nc.semaphore("dmi") as dmi,
        nc.semaphore("cp") as cp,
        nc.semaphore("g0") as g0,
        nc.semaphore("g1") as g1,
        nc.semaphore("g2") as g2,
        nc.semaphore("g3") as g3,
        nc.semaphore("v1") as v1,
        nc.semaphore("pc") as pc_sem,
        nc.semaphore("ck") as ck,
        nc.semaphore("od0") as od0,
        nc.semaphore("od1") as od1,
        nc.Block() as block,
    ):
        gsems = [g0, g1, g2, g3]
        NGS = len(gsems)
        odsems = [od0, od1]

        ncalls = len(calls)

        @block.sync
        def _(sync):
            sync.dma_start(IDX[:, :], idxB[:, :]).then_inc(dmi, 16)
            sync.dma_start(WV[:, :], wvB[:, :]).then_inc(dmi, 16)
            sync.dma_start(ID[:, :], idh[:, :]).then_inc(dmi, 16)
            for rep in range(reps):
                for j in range(NCH):
                    J = rep * NCH + j
                    sync.wait_ge(cp, J + 1)
                    sync.dma_start(accB[:, j * CHCOLS : (j + 1) * CHCOLS],
                                   OUT[:, J % 2, :]).then_inc(odsems[J % 2], 16)
            NJ = reps * NCH
            for s in range(2):
                tot = 16 * ((NJ - 1 - s) // 2 + 1) if NJ > s else 0
                if tot:
                    sync.wait_ge(odsems[s], tot)

        @block.scalar
        def _(scalar):
            for rep in range(reps):
                for j in range(NCH):
                    J = rep * NCH + j
                    scalar.wait_ge(ck, J + 1)
                    if J >= 2:
                        scalar.wait_ge(odsems[(J - 2) % 2], 16 * ((J - 2) // 2 + 1))
                    scalar.copy(OUT[:, J % 2, :], PS[:, J % 2, :])
                    scalar.drain().then_inc(cp, 1)

        @block.gpsimd
        def _(gpsimd):
            gpsimd.load_library(library_config.mlp)
            gpsimd.wait_ge(dmi, 48)
            for rep in range(reps):
                for i, call in enumerate(calls):
                    I = rep * ncalls + i
                    ntok = sum(n for (_, _, _, n, _) in call)
                    off = call_off[i]
                    if I >= NBUF:
                        gpsimd.wait_ge(pc_sem, I - NBUF + 1)
                    gpsimd.dma_gather(
                        G[:, I % NBUF, 0 : ntok // P, :],
                        ztab.rearrange("(a b) f -> a (b f)", b=2),
                        IDX[:, off // 16 : (off + ntok) // 16],
                        ntok, ntok, 2 * F, single_packet=False,
                    ).then_inc(gsems[I % NGS], 16)

        @block.vector
        def _(vector):
            for rep in range(reps):
                for i, call in enumerate(calls):
                    I = rep * ncalls + i
                    ntok = sum(n for (_, _, _, n, _) in call)
                    off = call_off[i]
                    vector.wait_ge(gsems[I % NGS], 16 * (I // NGS + 1))
                    wv_sl = WV[:, off // P : (off + ntok) // P, :]
                    vector.tensor_tensor(
                        G[:, I % NBUF, 0 : ntok // P, :].rearrange(
                            "p s (h f) -> p s h f", h=2),
                        G[:, I % NBUF, 0 : ntok // P, :].rearrange(
                            "p s (h f) -> p s h f", h=2),
                        bcast_free(wv_sl, F),
                        mybir.AluOpType.mult,
                    )
                    vector.drain().then_inc(v1, 1)

        @block.tensor
        def _(tensor):
            tensor.wait_ge(dmi, 48)
            for rep in range(reps):
                for i, call in enumerate(calls):
                    I = rep * ncalls + i
                    tensor.wait_ge(v1, I + 1)
                    for (j, r, roff, n, toff) in call:
                        J = rep * NCH + j
                        n_total = int(njr[j, r])
                        if r == 0 and roff == 0 and J >= 2:
                            tensor.wait_ge(cp, J - 1)
                        q0 = roff
                        while q0 < roff + n:
                            q1 = min(roff + n, (q0 // 1024 + 1) * 1024)
                            c0, c1 = q0 * F // P, q1 * F // P
                            s0 = (toff + q0 - roff) // P
                            s1 = (toff + q1 - roff) // P
                            bank = c0 // 512
                            stop = (last_touch[(j, bank)] == r) and (
                                q1 >= min(n_total, (bank + 1) * 1024))
                            tensor.matmul(PS[:, J % 2, c0:c1], ID[:, :],
                                          G[:, I % NBUF, s0:s1, 0:F],
                                          start=(r == 0), stop=False)
                            tensor.matmul(PS[:, J % 2, c0:c1], ID[:, :],
                                          G[:, I % NBUF, s0:s1, F : 2 * F],
                                          start=False, stop=stop)
                            if r == chunk_last_r[j] and q1 == n_total:
                                tensor.drain().then_inc(ck, 1)
                            q0 = q1
                    tensor.drain().then_inc(pc_sem, 1)

    nc.compile()
    return nc


def build_C(cfg, reps=1):
    nc = bacc.Bacc("TRN2")
    GP = cfg.GP
    NT = (cfg.NPCP + 511) // 512
    NTC = NT * 512

    ps_h = [nc.dram_tensor(f"p{i}", [P, GP * F], mybir.dt.float32, kind="ExternalInput")
            for i in range(NWIN)]
    dC = nc.dram_tensor("dC", [P, GP], mybir.dt.float32, kind="ExternalInput")
    W0h = nc.dram_tensor("W0h", [F, F], mybir.dt.float32, kind="ExternalInput")
    W0Th = nc.dram_tensor("W0Th", [F, F], mybir.dt.float32, kind="ExternalInput")
    WiTh = nc.dram_tensor("WiTh", [F, 3 * F], mybir.dt.float32, kind="ExternalInput")
    WhTh = nc.dram_tensor("WhTh", [F, 3 * F], mybir.dt.float32, kind="ExternalInput")
    biBh = nc.dram_tensor("biBh", [F, 3 * F], mybir.dt.float32, kind="ExternalInput")
    bhBh = nc.dram_tensor("bhBh", [F, 3 * F], mybir.dt.float32, kind="ExternalInput")
    pWTh = nc.dram_tensor("pWTh", [F, F], mybir.dt.float32, kind="ExternalInput")
    pbh = nc.dram_tensor("pbh", [F, 1], mybir.dt.float32, kind="ExternalInput")
    lWh = nc.dram_tensor("lWh", [F, 1], mybir.dt.float32, kind="ExternalInput")
    lbh = nc.dram_tensor("lbh", [1, 1], mybir.dt.float32, kind="ExternalInput")
    idh = nc.dram_tensor("idh", [P, P], mybir.dt.float32, kind="ExternalInput")
    yC = nc.dram_tensor("yC", [1, NTC], mybir.dt.float32, kind="ExternalOutput")

    from contextlib import ExitStack
    with ExitStack() as ctx:
        U = ctx.enter_context(nc.sbuf_tensor("U", [P, GP, F], mybir.dt.float32))
        PBUF = ctx.enter_context(nc.sbuf_tensor("PBUF", [P, GP, F], mybir.dt.float32))
        DINV = ctx.enter_context(nc.sbuf_tensor("DINV", [P, GP], mybir.dt.float32))
        ID = ctx.enter_context(nc.sbuf_tensor("ID", [P, P], mybir.dt.float32))
        W0s = ctx.enter_context(nc.sbuf_tensor("W0s", [F, F], mybir.dt.float32))
        W0Ts = ctx.enter_context(nc.sbuf_tensor("W0Ts", [F, F], mybir.dt.float32))
        WiTs = ctx.enter_context(nc.sbuf_tensor("WiTs", [F, 3 * F], mybir.dt.float32))
        WhTs = ctx.enter_context(nc.sbuf_tensor("WhTs", [F, 3 * F], mybir.dt.float32))
        biBs = ctx.enter_context(nc.sbuf_tensor("biBs", [F, 3 * F], mybir.dt.float32))
        bhBs = ctx.enter_context(nc.sbuf_tensor("bhBs", [F, 3 * F], mybir.dt.float32))
        pWTs = ctx.enter_context(nc.sbuf_tensor("pWTs", [F, F], mybir.dt.float32))
        pbs = ctx.enter_context(nc.sbuf_tensor("pbs", [F, 1], mybir.dt.float32))
        lWs = ctx.enter_context(nc.sbuf_tensor("lWs", [F, 1], mybir.dt.float32))
        lbs = ctx.enter_context(nc.sbuf_tensor("lbs", [1, 1], mybir.dt.float32))
        giS = ctx.enter_context(nc.sbuf_tensor("giS", [F, 3 * F], mybir.dt.float32))
        ghS = ctx.enter_context(nc.sbuf_tensor("ghS", [F, 3 * F], mybir.dt.float32))
        RZ = ctx.enter_context(nc.sbuf_tensor("RZ", [F, 2 * F], mybir.dt.float32))
        NN = ctx.enter_context(nc.sbuf_tensor("NN", [F, F], mybir.dt.float32))
        TMP = ctx.enter_context(nc.sbuf_tensor("TMP", [F, 3 * F], mybir.dt.float32))
        Wev = ctx.enter_context(nc.sbuf_tensor("Wev", [F, F], mybir.dt.float32))
        WTs = ctx.enter_context(nc.sbuf_tensor("WTs", [F, F], mybir.dt.float32))
        WcS = ctx.enter_context(nc.sbuf_tensor("WcS", [F, F], mybir.dt.float32))
        HT = ctx.enter_context(nc.sbuf_tensor("HT", [F, NTC], mybir.dt.float32))
        H1T = ctx.enter_context(nc.sbuf_tensor("H1T", [F, NTC], mybir.dt.float32))
        Y = ctx.enter_context(nc.sbuf_tensor("Y", [1, NTC], mybir.dt.float32))
        PG = ctx.enter_context(nc.psum_tensor("PG", [F, 2, 3 * F], mybir.dt.float32))
        PT = ctx.enter_context(nc.psum_tensor("PT", [F, 2, 512], mybir.dt.float32))
        PW = ctx.enter_context(nc.psum_tensor("PW", [F, F], mybir.dt.float32))
        PM = ctx.enter_context(nc.psum_tensor("PM", [F, 2, 512], mybir.dt.float32))
        PY = ctx.enter_context(nc.psum_tensor("PY", [1, 2, 512], mybir.dt.float32))
        dmi = ctx.enter_context(nc.semaphore("dmi"))
        u0d = ctx.enter_context(nc.semaphore("u0d"))
        pbd = ctx.enter_context(nc.semaphore("pbd"))
        prm = ctx.enter_context(nc.semaphore("prm"))
        ve = ctx.enter_context(nc.semaphore("ve"))
        pe = ctx.enter_context(nc.semaphore("pe"))
        ac = ctx.enter_context(nc.semaphore("ac"))
        block = ctx.enter_context(nc.Block())
        # ----- semaphore ledger (all static) -----
        # dmi: 16 per DMA.  Loads in order:
        #   1: p0->U, 2: p1->PBUF, 3: p2->PBUF, 4: p3->PBUF (serialized with ve)
        #   5: dC, 6: ID, 7: W0, 8: W0T, 9: WiT, 10: WhT, 11: biB, 12: bhB,
        #   13: pWT, 14: pb, 15: lW, 16: lb
        # ve: 1,2,3 = U+=p_i ; 4 = U*=dinv ; 5 = gi/gh bias+rz ; 6 = n_in ; 7 = Wev
        # pe: 1,2 = gi,gh ; 3 = Wev transpose ; 4 = Wc ; 5..4+GP = U transposes ;
        #     then NT main matmuls: 4+GP+t ; then NT head matmuls: 4+GP+NT+t
        # ac: 1 = sigmoid ; 2 = tanh ; 3 = WTs copy ; 4 = WcS copy ;
        #     5..4+GP = HT copies ; then NT relu: 4+GP+t ; then NT head: 4+GP+NT+t

        VE_R = 7
        PE_R = 4 + GP + 2 * NT
        AC_R = 4 + GP + 2 * NT

        def _c_load_params(sync):
            sync.dma_start(DINV[:, :], dC[:, :]).then_inc(prm, 16)
            sync.dma_start(ID[:, :], idh[:, :]).then_inc(prm, 16)
            sync.dma_start(W0s[:, :], W0h[:, :]).then_inc(prm, 16)
            sync.dma_start(W0Ts[:, :], W0Th[:, :]).then_inc(prm, 16)
            sync.dma_start(WiTs[:, :], WiTh[:, :]).then_inc(prm, 16)
            sync.dma_start(WhTs[:, :], WhTh[:, :]).then_inc(prm, 16)
            sync.dma_start(biBs[:, :], biBh[:, :]).then_inc(prm, 16)
            sync.dma_start(bhBs[:, :], bhBh[:, :]).then_inc(prm, 16)
            sync.dma_start(pWTs[:, :], pWTh[:, :]).then_inc(prm, 16)
            sync.dma_start(pbs[:, :], pbh[:, :]).then_inc(prm, 16)
            sync.dma_start(lWs[:, :], lWh[:, :]).then_inc(prm, 16)
            sync.dma_start(lbs[:, :], lbh[:, :]).then_inc(prm, 16)

        @block.sync
        def _(sync):
            for rep in range(reps):
                if rep > 0:
                    sync.wait_ge(pe, (rep - 1) * PE_R + 4 + GP)  # U transposes done
                sync.dma_start(U[:, :, :], ps_h[0].rearrange("p (g f) -> p g f", f=F)).then_inc(u0d, 16)
                for i in range(1, NWIN):
                    if rep > 0 or i > 1:
                        sync.wait_ge(ve, rep * VE_R + i - 1)
                    sync.dma_start(PBUF[:, :, :], ps_h[i].rearrange("p (g f) -> p g f", f=F)).then_inc(pbd, 16)
                if rep == 0:
                    _c_load_params(sync)
                sync.wait_ge(ac, rep * AC_R + AC_R)
                sync.dma_start(yC[:, :], Y[:, :]).then_inc(dmi, 16)
            sync.wait_ge(dmi, 16 * reps)

        @block.vector
        def _(vector):
            if NTC > cfg.NPCP:
                vector.memset(HT[:, cfg.NPCP :], 0.0)
            for rep in range(reps):
                vb = rep * VE_R
                pb = rep * PE_R
                ab = rep * AC_R
                vector.wait_ge(u0d, 16 * (rep + 1))
                for i in range(1, NWIN):
                    vector.wait_ge(pbd, 16 * (rep * (NWIN - 1) + i))
                    vector.tensor_tensor(U[:, :, :], U[:, :, :], PBUF[:, :, :],
                                         mybir.AluOpType.add)
                    vector.drain().then_inc(ve, 1)
                vector.wait_ge(prm, 16 * 12)
                vector.tensor_tensor(U[:, :, :], U[:, :, :], bcast_free(DINV[:, :], F),
                                     mybir.AluOpType.mult)
                vector.drain().then_inc(ve, 1)        # ve=vb+4
                vector.wait_ge(pe, pb + 2)
                vector.tensor_tensor(giS[:, :], PG[:, 0, :], biBs[:, :], mybir.AluOpType.add)
                vector.tensor_tensor(ghS[:, :], PG[:, 1, :], bhBs[:, :], mybir.AluOpType.add)
                vector.drain()
                vector.tensor_tensor(RZ[:, :], giS[:, 0 : 2 * F], ghS[:, 0 : 2 * F],
                                     mybir.AluOpType.add)
                vector.drain().then_inc(ve, 1)          # ve=vb+5
                vector.wait_ge(ac, ab + 1)
                vector.tensor_tensor(TMP[:, 0:F], RZ[:, 0:F], ghS[:, 2 * F : 3 * F],
                                     mybir.AluOpType.mult)
                vector.drain()
                vector.tensor_tensor(TMP[:, 0:F], giS[:, 2 * F : 3 * F], TMP[:, 0:F],
                                     mybir.AluOpType.add)
                vector.drain().then_inc(ve, 1)          # ve=vb+6
                vector.wait_ge(ac, ab + 2)
                vector.tensor_tensor(TMP[:, F : 2 * F], RZ[:, F : 2 * F], NN[:, :],
                                     mybir.AluOpType.mult)
                vector.tensor_tensor(TMP[:, 2 * F : 3 * F], RZ[:, F : 2 * F], W0s[:, :],
                                     mybir.AluOpType.mult)
                vector.drain()
                vector.tensor_tensor(Wev[:, :], NN[:, :], TMP[:, F : 2 * F],
                                     mybir.AluOpType.subtract)
                vector.drain()
                vector.tensor_tensor(Wev[:, :], Wev[:, :], TMP[:, 2 * F : 3 * F],
                                     mybir.AluOpType.add)
                vector.drain().then_inc(ve, 1)          # ve=vb+7

        @block.tensor
        def _(tensor):
            tensor.wait_ge(prm, 16 * 12)   # all params loaded
            for rep in range(reps):
                vb = rep * VE_R
                pb = rep * PE_R
                ab = rep * AC_R
                tensor.matmul(PG[:, 0, :], W0Ts[:, :], WiTs[:, :], start=True, stop=True).then_inc(pe, 1)
                tensor.matmul(PG[:, 1, :], W0Ts[:, :], WhTs[:, :], start=True, stop=True).then_inc(pe, 1)
                tensor.wait_ge(ve, vb + 7)
                tensor.transpose(PW[:, :], Wev[:, :], ID[0:F, 0:F]).then_inc(pe, 1)   # pe=pb+3
                tensor.wait_ge(ac, ab + 3)          # WTs in sbuf (PW free again)
                tensor.matmul(PW[:, :], WTs[:, :], pWTs[:, :], start=True, stop=True).then_inc(pe, 1)  # pe=pb+4
                tensor.wait_ge(ve, vb + 4)          # U final
                for g in range(GP):
                    if rep * GP + g >= 2:
                        tensor.wait_ge(ac, ab + 4 + g - 1) if g >= 2 else tensor.wait_ge(ac, (rep - 1) * AC_R + 4 + GP)
                    tensor.transpose(PT[:, g % 2, 0:P], U[:, g, :], ID[:, :]).then_inc(pe, 1)  # pe=pb+4+g+1
                tensor.wait_ge(ac, ab + 4 + GP)     # HT fully assembled + WcS copied
                for t in range(NT):
                    if t >= 2:
                        tensor.wait_ge(ac, ab + 4 + GP + t - 1)
                    elif rep > 0:
                        tensor.wait_ge(ac, (rep - 1) * AC_R + 4 + GP + NT)
                    tensor.matmul(PM[:, t % 2, :], WcS[:, :], HT[:, t * 512 : (t + 1) * 512],
                                  start=True, stop=True).then_inc(pe, 1)   # pe=pb+4+GP+t+1
                tensor.wait_ge(ac, ab + 4 + GP + NT)   # all relus done (H1T complete)
                for t in range(NT):
                    if t >= 2:
                        tensor.wait_ge(ac, ab + 4 + GP + NT + t - 1)
                    elif rep > 0:
                        tensor.wait_ge(ac, (rep - 1) * AC_R + AC_R)
                    tensor.matmul(PY[:, t % 2, :], lWs[:, :], H1T[:, t * 512 : (t + 1) * 512],
                                  start=True, stop=True).then_inc(pe, 1)

        @block.scalar
        def _(scalar):
            for rep in range(reps):
                vb = rep * VE_R
                pb = rep * PE_R
                ab = rep * AC_R
                scalar.wait_ge(ve, vb + 5)
                scalar.activation(RZ[:, :], RZ[:, :], mybir.ActivationFunctionType.Sigmoid)
                scalar.drain().then_inc(ac, 1)
                scalar.wait_ge(ve, vb + 6)
                scalar.activation(NN[:, :], TMP[:, 0:F], mybir.ActivationFunctionType.Tanh)
                scalar.drain().then_inc(ac, 1)  # ac=ab+2
                scalar.wait_ge(pe, pb + 3)
                scalar.copy(WTs[:, :], PW[:, 0:F])
                scalar.drain().then_inc(ac, 1)     # ac=ab+3
                scalar.wait_ge(pe, pb + 4)
                scalar.copy(WcS[:, :], PW[:, 0:F])
                scalar.drain().then_inc(ac, 1)     # ac=ab+4
                for g in range(GP):
                    scalar.wait_ge(pe, pb + 4 + g + 1)
                    scalar.copy(HT[:, g * P : (g + 1) * P], PT[:, g % 2, 0:P])
                    scalar.drain().then_inc(ac, 1)  # ac=ab+4+g+1
                for t in range(NT):
                    scalar.wait_ge(pe, pb + 4 + GP + t + 1)
                    scalar.activation(H1T[:, t * 512 : (t + 1) * 512], PM[:, t % 2, :],
                                      mybir.ActivationFunctionType.Relu,
                                      bias=pbs[:, 0:1])
                    scalar.drain().then_inc(ac, 1)
                for t in range(NT):
                    scalar.wait_ge(pe, pb + 4 + GP + NT + t + 1)
                    if rep > 0 and t == 0:
                        scalar.wait_ge(dmi, 16 * rep)   # prev yC DMA done before Y overwrite
                    scalar.activation(Y[:, t * 512 : (t + 1) * 512], PY[:, t % 2, :],
                                      mybir.ActivationFunctionType.Identity,
                                      bias=lbs[0:1, 0:1])
                    scalar.drain().then_inc(ac, 1)

    nc.compile()
    return nc


def make_A_inputs(x, wA_list, cfg):
    ins = []
    for c in range(NC):
        xA = np.zeros((cfg.NPCP, F), np.float32)
        xA[: cfg.NPC] = x[c * cfg.NPC : (c + 1) * cfg.NPC]
        ins.append({"xA": grid_layout(xA, cfg), "wA": wA_list[c]})
    return ins


def assemble_after_A(resA, cfg):
    """Collect z (natural order) and per-core dinv grids."""
    N = cfg.N
    z = np.zeros((N, F), np.float32)
    dinv_grids = []
    for c in range(NC):
        zg = ungrid_layout(np.asarray(resA[c]["zA"]), cfg, F)
        z[c * cfg.NPC : (c + 1) * cfg.NPC] = zg[: cfg.NPC]
        dinv_grids.append(np.asarray(resA[c]["dA"]))
    return z, dinv_grids


def make_B_inputs(z, sched, cfg):
    ident = np.eye(P, dtype=np.float32)
    ins = []
    for pc in sched["per_core"]:
        wno = pc["win"]
        ztab = np.ascontiguousarray(z[wno::NWIN].astype(np.float32))
        assert ztab.shape == (cfg.WINN, F)
        ins.append({
            "ztab": ztab,
            "idxB": sched["idx_arrs"][pc["cid"]],
            "wvB": sched["wv_arrs"][pc["cid"]],
            "idh": ident,
        })
    return ins


def make_C_inputs(resB, dinv_grids, sched, cfg, params):
    """Reorder B partials into per-C-core natural-layout grids."""
    N = cfg.N
    # slot-value grids per B core: [SLOTS] -> value[64]
    grids = []
    for bc in range(NC):
        a = np.asarray(resB[bc]["accB"])  # [128, SLOTS*F//128]
        g = a.reshape(P, sched["SLOTS"] // P, F)
        grids.append(g)

    # per half: node -> hidx
    node_hidx = np.full((NHALF, N), -1, dtype=np.int64)
    half_nodes_l = []
    for k in range(NHALF):
        hn = sched["per_core"][k * NWIN]["half_nodes"]
        node_hidx[k, hn] = np.arange(len(hn))
        half_nodes_l.append(hn)

    (W0, gru_Wi, gru_Wh, gru_bi, gru_bh, proj_W, proj_b, lin_W, lin_b) = params
    ident = np.eye(P, dtype=np.float32)
    common = {
        "W0h": np.ascontiguousarray(W0.astype(np.float32)),
        "W0Th": np.ascontiguousarray(W0.T.astype(np.float32)),
        "WiTh": np.ascontiguousarray(gru_Wi.T.astype(np.float32)),
        "WhTh": np.ascontiguousarray(gru_Wh.T.astype(np.float32)),
        "biBh": np.ascontiguousarray(np.tile(gru_bi[None, :], (F, 1)).astype(np.float32)),
        "bhBh": np.ascontiguousarray(np.tile(gru_bh[None, :], (F, 1)).astype(np.float32)),
        "pWTh": np.ascontiguousarray(proj_W.T.astype(np.float32)),
        "pbh": np.ascontiguousarray(proj_b[:, None].astype(np.float32)),
        "lWh": np.ascontiguousarray(lin_W.T.astype(np.float32).reshape(F, 1)),
        "lbh": np.ascontiguousarray(lin_b.astype(np.float32).reshape(1, 1)),
        "idh": ident,
    }

    ins = []
    for c in range(NC):
        vs = np.arange(c * cfg.NPC, (c + 1) * cfg.NPC)
        kk = (vs // NWIN) % NHALF
        d = dict(common)
        for wno in range(NWIN):
            vals = np.zeros((cfg.NPCP, F), np.float32)
            for k in range(NHALF):
                m = kk == k
                if not m.any():
                    continue
                bc = k * NWIN + wno
                pc = sched["per_core"][bc]
                hidx = node_hidx[k, vs[m]]
                slots = pc["node_slot"][hidx]
                vals[np.nonzero(m)[0]] = grids[bc][slots % P, slots // P, :]
            d[f"p{wno}"] = grid_layout(vals, cfg)
        d["dC"] = dinv_grids[c]
        ins.append(d)
    return ins


def assemble_y(resC, cfg):
    N = cfg.N
    y = np.zeros((N, 1), np.float32)
    for c in range(NC):
        yc = np.asarray(resC[c]["yC"]).reshape(-1)
        y[c * cfg.NPC : (c + 1) * cfg.NPC, 0] = yc[: cfg.NPC]
    return y


# ---------------------------------------------------------------------------
# Top-level kernel entry
# ---------------------------------------------------------------------------

_CACHE = {}


def _get_programs(src, dst, w, cfg):
    key = hash((src.tobytes(), dst.tobytes()))
    if key not in _CACHE:
        RA, wA_list = prep_A(src, dst, w, cfg)
        sched = prep_B(src, dst, w, cfg)
        ncA = build_A(cfg, RA)
        ncB = build_B(cfg, sched)
        ncC = build_C(cfg)
        _CACHE.clear()
        _CACHE[key] = (RA, wA_list, sched, ncA, ncB, ncC)
    return _CACHE[key]


def kernel(x, edge_index, edge_weight, W0, gru_Wi, gru_Wh, gru_bi, gru_bh,
           proj_W, proj_b, lin_W, lin_b):
    from concourse.bass_utils import run_bass_kernel_spmd

    x = np.asarray(x, dtype=np.float32)
    ei = np.asarray(edge_index)
    src = ei[0].astype(np.int64)
    dst = ei[1].astype(np.int64)
    w = np.asarray(edge_weight, dtype=np.float32)
    params = (np.asarray(W0), np.asarray(gru_Wi), np.asarray(gru_Wh),
              np.asarray(gru_bi), np.asarray(gru_bh), np.asarray(proj_W),
              np.asarray(proj_b), np.asarray(lin_W), np.asarray(lin_b))

    cfg = Cfg(N=x.shape[0])
    RA, wA_list, sched, ncA, ncB, ncC = _get_programs(src, dst, w, cfg)
    cores = list(range(NC))

    insA = make_A_inputs(x, wA_list, cfg)
    resA = run_bass_kernel_spmd(ncA, insA, cores).results
    z, dinv_grids = assemble_after_A(resA, cfg)

    insB = make_B_inputs(z, sched, cfg)
    resB = run_bass_kernel_spmd(ncB, insB, cores).results

    insC = make_C_inputs(resB, dinv_grids, sched, cfg, params)
    resC = run_bass_kernel_spmd(ncC, insC, cores).results
    return assemble_y(resC, cfg)

